# revision 10
# baseline (speedup 1.0000x reference)
"""CACE-A edge message passing on 8 Trainium2 NeuronCores.

Strategy (receiver-sharded data parallel):
  - Nodes are striped across 8 cores (1250 nodes/core, padded to 10 blocks
    of 128).  Every edge is routed to the core that owns its *receiver*
    node, so the segment-sum is fully local — no collectives.
  - Host does index work only: sort edges by receiver, pack per-edge
    records (d, u, sender-embedding, local node id) into fixed 128-edge
    chunks per 128-node block.
  - Device, per chunk of 128 edges: build the factored per-edge message
      m[e, (i, r, c1)] = ang_i(u) * exp(-d^2/2w_r^2)*env(d) * W_send[t_e, c1]
    (320 floats) and a one-hot selection matrix onehot[e, n_local]
    (iota compare), then scatter-sum via the TensorEngine:
      B[n, 320] += onehot^T @ m      (PSUM accumulation over chunks)
  - Per block: radial mixing F[n,(i,b,c1)] = sum_r B[n,(i,r,c1)] W_rad[l_i,r,b]
    (DVE mult+reduce), then multiply by the receiver embedding (c2) while
    interleaving directly into the a0/a1/a2 output layouts, DMA out.
"""

import os
from contextlib import ExitStack

import numpy as np

import concourse.bass as bass
import concourse.tile as tile
from concourse import bacc, mybir
from concourse.bass_utils import run_bass_kernel_spmd

F32 = mybir.dt.float32
AF = mybir.ActivationFunctionType
ALU = mybir.AluOpType

# ---- problem constants (nn_CaceA_79096117723650) ----
N, E = 10000, 100000
NCORES = 8
NPC = N // NCORES            # 1250 nodes per core
P = 128
NBLK = 10                    # node blocks/core, 10*128=1280 >= 1250
NRBF, NRAD, NANG = 8, 12, 10
CUT = 4.0
ZS = np.array([1, 6, 7, 8, 9])
# l of each angular index; i-groups are contiguous: i=0 (l=0), i=1..3 (l=1), i=4..9 (l=2)
LGROUPS = ((0, 0, 1), (1, 1, 3), (2, 4, 6))   # (l, i0, ni)
NF = 12                      # floats per edge record: d,u0,u1,u2,s0..3,loc,pad*3
NPAY = NANG * NRBF * 4       # 320: payload idx p = i*32 + r*4 + c1
NQ = NANG * NRAD * 4         # 480: q = i*48 + b*4 + c1
D_OUT = NRAD * 16            # 192

MM_DT = mybir.dt.float32r    # matmul compute dtype (bitcast view of f32)

_CACHE = {}


def build_nc(cpb: int) -> bass.Bass:
    """Build the single-core Bass program (SPMD across 8 cores)."""
    nchunks = NBLK * cpb
    nc = bacc.Bacc("TRN2", target_bir_lowering=False, debug=False)

    edata = nc.declare_dram_parameter("edata", [P, nchunks * NF], F32, isOutput=False)
    er_d = nc.declare_dram_parameter("er", [NBLK, P, 4], F32, isOutput=False)
    wrep_d = nc.declare_dram_parameter("wrep", [P, 3 * NRAD * NRBF], F32, isOutput=False)
    aneg_d = nc.declare_dram_parameter("aneg", [P, NRBF], F32, isOutput=False)
    out0 = nc.declare_dram_parameter("out0", [NBLK * P, D_OUT], F32, isOutput=True)
    out1 = nc.declare_dram_parameter("out1", [NBLK * P, D_OUT * 3], F32, isOutput=True)
    out2 = nc.declare_dram_parameter("out2", [NBLK * P, D_OUT * 9], F32, isOutput=True)

    with tile.TileContext(nc) as tc, ExitStack() as ctx:
        consts = ctx.enter_context(tc.tile_pool(name="consts", bufs=1))
        edp = ctx.enter_context(tc.tile_pool(name="edp", bufs=2))
        small = ctx.enter_context(tc.tile_pool(name="small", bufs=2))
        mpool = ctx.enter_context(tc.tile_pool(name="mpool", bufs=4))
        ohpool = ctx.enter_context(tc.tile_pool(name="ohpool", bufs=4))
        bigp = ctx.enter_context(tc.tile_pool(name="bigp", bufs=2))
        outp = ctx.enter_context(tc.tile_pool(name="outp", bufs=2))
        psum = ctx.enter_context(tc.tile_pool(name="psum", bufs=4, space="PSUM"))

        wrep_t = consts.tile([P, 3 * NRAD * NRBF], F32)
        nc.sync.dma_start(wrep_t[:], wrep_d[:])
        aneg_t = consts.tile([P, NRBF], F32)
        nc.sync.dma_start(aneg_t[:], aneg_d[:])
        iota_i = consts.tile([P, P], mybir.dt.int32)
        nc.gpsimd.iota(iota_i[:], pattern=[[1, P]], base=0, channel_multiplier=0)
        iota_f = consts.tile([P, P], F32)
        nc.vector.tensor_copy(iota_f[:], iota_i[:])

        for b in range(NBLK):
            ed_t = edp.tile([P, cpb * NF], F32)
            nc.sync.dma_start(ed_t[:], edata[:, b * cpb * NF:(b + 1) * cpb * NF])
            edv = ed_t[:].rearrange("p (c f) -> p c f", c=cpb)
            er_t = small.tile([P, 4], F32, tag="er")
            nc.sync.dma_start(er_t[:], er_d[b])

            d_ap = edv[:, :, 0]                     # [P, cpb]
            # env(x) = 1 + x^5 (-21 + 35x - 15x^2),  x = d/rc   (= 0 at x=1)
            x = small.tile([P, cpb], F32, tag="x")
            nc.vector.tensor_scalar_mul(x[:], d_ap, 1.0 / CUT)
            dd = small.tile([P, cpb], F32, tag="dd")
            nc.vector.tensor_tensor(dd[:], d_ap, d_ap, op=ALU.mult)
            x2 = small.tile([P, cpb], F32, tag="x2")
            nc.vector.tensor_tensor(x2[:], x[:], x[:], op=ALU.mult)
            x4 = small.tile([P, cpb], F32, tag="x4")
            nc.vector.tensor_tensor(x4[:], x2[:], x2[:], op=ALU.mult)
            x5 = small.tile([P, cpb], F32, tag="x5")
            nc.vector.tensor_tensor(x5[:], x4[:], x[:], op=ALU.mult)
            t1 = small.tile([P, cpb], F32, tag="t1")
            nc.vector.tensor_scalar(t1[:], x[:], -15.0, 35.0, op0=ALU.mult, op1=ALU.add)
            t2 = small.tile([P, cpb], F32, tag="t2")
            nc.vector.tensor_tensor(t2[:], t1[:], x[:], op=ALU.mult)
            t3 = small.tile([P, cpb], F32, tag="t3")
            nc.vector.tensor_scalar_add(t3[:], t2[:], -21.0)
            e5 = small.tile([P, cpb], F32, tag="e5")
            nc.vector.tensor_tensor(e5[:], t3[:], x5[:], op=ALU.mult)
            env = small.tile([P, cpb], F32, tag="env")
            nc.vector.tensor_scalar_add(env[:], e5[:], 1.0)

            # s*env  [P, cpb, 4]
            se = small.tile([P, cpb * 4], F32, tag="se")
            sev = se[:].rearrange("p (c q) -> p c q", c=cpb)
            nc.vector.tensor_tensor(
                sev, edv[:, :, 4:8],
                env[:].unsqueeze(2).to_broadcast([P, cpb, 4]), op=ALU.mult)

            Bp = psum.tile([P, NPAY], F32)
            for ci in range(cpb):
                # radial basis: f_r = exp(aneg_r * d^2)
                fa = small.tile([P, NRBF], F32, tag="fa")
                nc.vector.tensor_tensor(
                    fa[:], aneg_t[:],
                    dd[:, ci:ci + 1].to_broadcast([P, NRBF]), op=ALU.mult)
                f_t = small.tile([P, NRBF], F32, tag="f")
                nc.scalar.activation(f_t[:], fa[:], AF.Exp)
                # angular monomials [1, x, y, z, xx, xy, xz, yy, yz, zz]
                ang = small.tile([P, NANG], F32, tag="ang")
                nc.vector.memset(ang[:, 0:1], 1.0)
                u_ap = edv[:, ci, 1:4]
                nc.vector.tensor_copy(ang[:, 1:4], u_ap)
                nc.vector.tensor_scalar_mul(ang[:, 4:7], u_ap, edv[:, ci, 1:2])
                nc.vector.tensor_scalar_mul(ang[:, 7:9], edv[:, ci, 2:4], edv[:, ci, 2:3])
                nc.vector.tensor_scalar_mul(ang[:, 9:10], edv[:, ci, 3:4], edv[:, ci, 3:4])
                # fs[e,(r,c1)] = f_r * (s*env)_c1
                fs = small.tile([P, NRBF * 4], F32, tag="fs")
                fsv = fs[:].rearrange("p (r c) -> p r c", r=NRBF)
                nc.vector.tensor_tensor(
                    fsv, f_t[:].unsqueeze(2).to_broadcast([P, NRBF, 4]),
                    sev[:, ci, :].unsqueeze(1).to_broadcast([P, NRBF, 4]),
                    op=ALU.mult)
                # m[e, (i, r, c1)] = ang_i * fs
                m_t = mpool.tile([P, NPAY], MM_DT)
                mv = m_t[:].rearrange("p (i rc) -> p i rc", i=NANG)
                nc.vector.tensor_tensor(
                    mv, fs[:].unsqueeze(1).to_broadcast([P, NANG, NRBF * 4]),
                    ang[:].unsqueeze(2).to_broadcast([P, NANG, NRBF * 4]),
                    op=ALU.mult)
                # one-hot local receiver matrix
                oh = ohpool.tile([P, P], MM_DT)
                nc.vector.tensor_tensor(
                    oh[:], edv[:, ci, 8:9].to_broadcast([P, P]), iota_f[:],
                    op=ALU.is_equal)
                # scatter-sum: B[n, :] += onehot^T @ m
                nc.tensor.matmul(Bp[:], oh[:], m_t[:],
                                 start=(ci == 0), stop=(ci == cpb - 1))

            # radial mixing: F[n, (i,b,c1)] = sum_r B[n,(i,r,c1)] * W[l_i, r, b]
            p5 = bigp.tile([P, NQ * NRBF], F32, tag="p5")
            lof = (0, 1, 1, 1, 2, 2, 2, 2, 2, 2)
            for i in range(NANG):
                l = lof[i]
                bv = (Bp[:, i * 32:(i + 1) * 32]
                      .rearrange("p (r c) -> p r c", r=NRBF)
                      .unsqueeze(1).to_broadcast([P, NRAD, NRBF, 4])
                      .transpose([0, 1, 3, 2]))
                wv = (wrep_t[:, l * 96:(l + 1) * 96]
                      .rearrange("p (b r) -> p b r", b=NRAD)
                      .unsqueeze(2).to_broadcast([P, NRAD, 4, NRBF]))
                pv = (p5[:, i * 384:(i + 1) * 384]
                      .rearrange("p (b c r) -> p b c r", b=NRAD, c=4, r=NRBF))
                nc.vector.tensor_tensor(pv, bv, wv, op=ALU.mult)
            f_q = bigp.tile([P, NQ], F32, tag="fq")
            nc.vector.tensor_reduce(
                f_q[:], p5[:].rearrange("p (q r) -> p q r", q=NQ),
                axis=mybir.AxisListType.X, op=ALU.add)

            # output stage: multiply by receiver embedding (c2) + interleave
            def er_bc(shape):
                v = er_t[:].unsqueeze(1).unsqueeze(1)     # [P,1,1,4]
                while len(shape) > v.ndim:
                    v = v.unsqueeze(v.ndim)
                return v.to_broadcast(shape)

            a0_t = outp.tile([P, D_OUT], F32, tag="a0")
            in0 = (f_q[:, 0:48].rearrange("p (b c) -> p b c", b=NRAD)
                   .unsqueeze(3).to_broadcast([P, NRAD, 4, 4]))
            nc.vector.tensor_tensor(
                a0_t[:].rearrange("p (b c k) -> p b c k", b=NRAD, c=4),
                in0, er_bc([P, NRAD, 4, 4]), op=ALU.mult)

            a1_t = outp.tile([P, D_OUT * 3], F32, tag="a1")
            a1v = a1_t[:].rearrange("p (b c k j) -> p b c k j", b=NRAD, c=4, k=4)
            for j in range(3):
                in0 = (f_q[:, (1 + j) * 48:(2 + j) * 48]
                       .rearrange("p (b c) -> p b c", b=NRAD)
                       .unsqueeze(3).to_broadcast([P, NRAD, 4, 4]))
                nc.vector.tensor_tensor(
                    a1v[:, :, :, :, j], in0, er_bc([P, NRAD, 4, 4]), op=ALU.mult)

            a2_t = outp.tile([P, D_OUT * 9], F32, tag="a2")
            a2v = a2_t[:].rearrange("p (b c k q) -> p b c k q", b=NRAD, c=4, k=4)
            # a2[n, d, j, k] = F[i2[j,k], d]*er; i2 = [[4,5,6],[5,7,8],[6,8,9]]
            i2 = ((4, 5, 6), (5, 7, 8), (6, 8, 9))
            for j in range(3):
                for k in range(3):
                    i = i2[j][k]
                    in0 = (f_q[:, i * 48:(i + 1) * 48]
                           .rearrange("p (b c) -> p b c", b=NRAD)
                           .unsqueeze(3).to_broadcast([P, NRAD, 4, 4]))
                    nc.vector.tensor_tensor(
                        a2v[:, :, :, :, j * 3 + k], in0,
                        er_bc([P, NRAD, 4, 4]), op=ALU.mult)

            nc.sync.dma_start(out0[b * P:(b + 1) * P, :], a0_t[:])
            nc.sync.dma_start(out1[b * P:(b + 1) * P, :], a1_t[:])
            nc.sync.dma_start(out2[b * P:(b + 1) * P, :], a2_t[:])

    nc.compile()
    return nc


def pack_inputs(atomic_numbers, edge_index, dij, uij, W_send, W_recv, widths, W_rad):
    """Host-side sharding/packing. Index manipulation + embedding lookups only."""
    an = np.asarray(atomic_numbers)
    ei = np.asarray(edge_index)
    d = np.asarray(dij, dtype=np.float32)
    u = np.asarray(uij, dtype=np.float32)
    Ws = np.asarray(W_send, dtype=np.float32)
    Wr = np.asarray(W_recv, dtype=np.float32)
    w = np.asarray(widths, dtype=np.float32)
    Wrad = np.asarray(W_rad, dtype=np.float32)

    z_idx = np.searchsorted(ZS, an)
    src, dst = ei[0], ei[1]
    s_e = Ws[z_idx[src]]                           # [E, 4] sender embedding
    er_n = Wr[z_idx]                               # [N, 4] receiver embedding

    order = np.argsort(dst, kind="stable")
    dst_s, src_s = dst[order], src[order]
    d_s, u_s, s_s = d[order], u[order], s_e[order]

    core = dst_s // NPC
    off = dst_s - core * NPC
    blk = off // P
    loc = off - blk * P
    gblk = core * NBLK + blk                       # globally sorted since dst sorted
    bounds = np.searchsorted(gblk, np.arange(NCORES * NBLK + 1))
    counts = np.diff(bounds)
    cpb = int(np.ceil(counts.max() / P))
    rank = np.arange(E) - bounds[gblk]
    chunk = rank // P
    lane = rank - chunk * P

    nchunks = NBLK * cpb
    edata = np.zeros((NCORES, P, nchunks, NF), dtype=np.float32)
    edata[:, :, :, 0] = CUT                        # pad edges: d=CUT -> env=0
    cidx = blk * cpb + chunk
    edata[core, lane, cidx, 0] = d_s
    edata[core, lane, cidx, 1:4] = u_s
    edata[core, lane, cidx, 4:8] = s_s
    edata[core, lane, cidx, 8] = loc.astype(np.float32)

    er_full = np.zeros((NCORES, NBLK, P, 4), dtype=np.float32)
    ner = NBLK * P
    for k in range(NCORES):
        rows = er_n[k * NPC:(k + 1) * NPC]
        er_full[k].reshape(ner, 4)[: rows.shape[0]] = rows

    # wrep row: idx l*96 + b*8 + r  ->  W_rad[l, r, b]
    wrep = np.tile(Wrad.transpose(0, 2, 1).reshape(1, -1), (P, 1)).astype(np.float32)
    aneg = np.tile((-0.5 / (w * w)).reshape(1, -1), (P, 1)).astype(np.float32)

    in_maps = [
        dict(edata=np.ascontiguousarray(edata[k].reshape(P, nchunks * NF)),
             er=np.ascontiguousarray(er_full[k]),
             wrep=wrep, aneg=aneg)
        for k in range(NCORES)
    ]
    return in_maps, cpb


def assemble(results):
    a0 = np.concatenate([np.asarray(r["out0"])[:NPC] for r in results], axis=0)
    a1 = np.concatenate([np.asarray(r["out1"])[:NPC] for r in results], axis=0)
    a2 = np.concatenate([np.asarray(r["out2"])[:NPC] for r in results], axis=0)
    return (a0.astype(np.float32),
            a1.reshape(N, D_OUT, 3).astype(np.float32),
            a2.reshape(N, D_OUT, 3, 3).astype(np.float32))


def kernel(atomic_numbers, edge_index, dij, uij, positions,
           W_send, W_recv, widths, W_rad):
    in_maps, cpb = pack_inputs(atomic_numbers, edge_index, dij, uij,
                               W_send, W_recv, widths, W_rad)
    key = ("nc", cpb)
    if key not in _CACHE:
        _CACHE[key] = build_nc(cpb)
    nc = _CACHE[key]
    trace = bool(int(os.environ.get("KERNEL_TRACE", "0")))
    res = run_bass_kernel_spmd(nc, in_maps, list(range(NCORES)), trace=trace)
    kernel.last_run = res
    return assemble(res.results)


kernel.last_run = None
